# revision 1
# baseline (speedup 1.0000x reference)
"""Custom GRU cell kernel for Trainium2, data-parallel over batch on 8 NeuronCores.

Latency-optimized recurrence: total time ~= T * L where L is the serial
per-step dependency chain, so the design minimizes L. Two independent batch
substreams of 128 columns run concurrently per core (each engine has ~50%
slack per chain), so the full 256-column batch advances one step per L.

Key algebraic trick: h_t = m2_t - m1_t with m1_t = (uhat_t - 1)*h_{t-1}
(ready before tanh) and m2_t = uhat_t * htil_t (right after tanh). The
r-gate - the chain-critical input of the next step - is computed as
U_r h_t = U_r m2_t + (-U_r) m1_t, so sigma_r waits only one matmul after m2
instead of tanh -> m2 -> h_t -> U_r h_t. The z and mmh gates are off the
sigma_r chain and use plain U @ h_{t-1} (fewer PE instructions; the PE
sequencer dispatch cost scales with instruction count).

Chain per step: m2 (DVE) -> U_r m2 / -U_r m1 (PE) -> sigma_r (ACT) ->
t1 = r*mmh (DVE, PSUM src) -> ident-MM folds t1 into xh (PE) -> tanh (ACT)
-> m2 (DVE).

The x-side matmuls are batched across steps (TC=8, divisible by 4, so
batches never straddle chunks): the r and xh gates are QUAD-batched (one
N=512 matmul per gate per substream covering 4 steps, filling a [U,512]
PSUM bank), and the shared z gate is PAIR-batched (one N=512 full-width
matmul per 2 steps - a quad would need 2 banks). Step t's sigma/tanh read
their quarter/half of the batch bank mid-accumulation-group (consecutive
steps' reads are a full chain-period apart, so the bank overlap tracker
costs nothing; skip_group_check silences the simulator's conservative
mid-group-read error - validated correct on hardware). This brings the
kernel to ~24 PE instructions per step; TC=8 also shortens the pipeline
ramp vs TC=20 (sim-swept).

PSUM: four single-reader banks per substream (8 banks total, bufs=1), one
accumulation group per bank per step (start on first matmul, stop on last,
single read after the stop - the bank-overlap tracker then never serializes
independent reads and CoreSim group rules hold):
  pr [128,128] r (read by sigma_r), pz z (sigma_u),
  pmm mmh (t1), pxh xh (tanh).
The z gate is shared across substreams: one full-width W_z x matmul into a
shared z bank and one fused FD=256 sigma_u per step (fewer PE instructions
and ACT ops; the z gate is off the critical chain so the substream coupling
it introduces is tolerable). uhat/m1 are computed in the H2 phase after the
fused sigma_u. `a` is host-broadcast to [128, T, BL] (arep) so uhat = u * a_t
is a bf16 SBUF 2x op; m1 = (uhat-1)*h is a DVE scalar_tensor_tensor (the
Pool engine does not support STT); gpsimd measured slower for h_t, so
everything elementwise stays on DVE. State h is bf16, written into the
output chunk ([U, TC, BL]) and DMA'd out per chunk; matmul inputs bf16,
PSUM f32.

Rejected with measurements: sharing the r bank the same way (sim +58us from
bank read-serialization on the chain-critical sigma_r), splitting all three
gates m2/m1 (doubles PE instruction count; HW regressed), gpsimd for
elementwise (HW ~+250ns/op serial impact), per-substream sigma_u (one more
ACT op in the queue ahead of the chain's tanh).
"""

import sys

sys.path.insert(0, "/opt/trn_rl_repo")

import numpy as np
import ml_dtypes

import concourse.bass as bass  # noqa: F401  (import registers rust bindings)
import concourse.mybir as mybir
import concourse.tile as tile
from concourse.tile import add_dep_helper
from concourse import bacc
from concourse.bass_utils import run_bass_kernel_spmd

BF16 = mybir.dt.bfloat16
F32 = mybir.dt.float32
AF = mybir.ActivationFunctionType
OP = mybir.AluOpType

B, T, U = 2048, 200, 128
NCORES = 8
BL = B // NCORES  # 256 batch rows per core
NS = 2  # substreams per core
SW = BL // NS  # 128 batch columns per substream
TC = 8  # timesteps per chunk (div by 4: r/xh x-side matmuls are quad-batched)
NCHUNK = T // TC

M1_ENGINE = "dve"  # "dve" (STT unsupported on Pool engine)
FUSE_SIGMA = False  # fuse sigma over [r|z] (shorter ACT busy, longer chain)

PROFILE = False
LAST_RESULT = None
LAST_IN_MAPS = None

_cache = {}


def _build(has_brz: bool, T_=T, TC_=TC, BL_=BL, reps=1):
    NCHUNK_ = T_ // TC_
    nc = bacc.Bacc("TRN2", target_bir_lowering=False)

    xt = nc.dram_tensor("xt", [U, T_, BL_], BF16, kind="ExternalInput")
    arep = nc.dram_tensor("arep", [U, T_, BL_], BF16, kind="ExternalInput")
    h0t = nc.dram_tensor("h0t", [U, BL_], BF16, kind="ExternalInput")
    # wcat: W_r, U_r, W_z, U_z, W_h, U_h, -U_r, -U_z, -U_h
    wcat = nc.dram_tensor("wcat", [9, U, U], BF16, kind="ExternalInput")
    ident_d = nc.dram_tensor("ident", [U, U], BF16, kind="ExternalInput")
    biases = nc.dram_tensor("biases", [U, 3], F32, kind="ExternalInput")
    outt = nc.dram_tensor("outt", [U, T_, BL_], BF16, kind="ExternalOutput")

    with tile.TileContext(nc) as tc:
        with (
            tc.tile_pool(name="const", bufs=1) as cpool,
            tc.tile_pool(name="xchunk", bufs=2) as xpool,
            tc.tile_pool(name="achunk", bufs=2) as apool,
            tc.tile_pool(name="ochunk", bufs=3) as opool,
            tc.tile_pool(name="work", bufs=4) as wpool,
            tc.tile_pool(name="ppr", bufs=1, space="PSUM") as prpool,
            tc.tile_pool(name="pmm", bufs=1, space="PSUM") as pmmpool,
            tc.tile_pool(name="ppz", bufs=1, space="PSUM") as pzpool,
            tc.tile_pool(name="pxh", bufs=1, space="PSUM") as pxhpool,
        ):
            wts = []
            for i in range(9):
                wt = cpool.tile([U, U], BF16, tag=f"w{i}")
                nc.sync.dma_start(wt[:], wcat[i])
                wts.append(wt)
            w_r, u_r, w_z, u_z, w_h, u_h, un_r, un_z, un_h = wts
            ident = cpool.tile([U, U], BF16, tag="ident")
            nc.sync.dma_start(ident[:], ident_d[:])
            btile = cpool.tile([U, 3], F32, tag="biases")
            nc.sync.dma_start(btile[:], biases[:])
            b_r_ap = btile[:, 0:1]
            b_z_ap = btile[:, 1:2]
            b_h_ap = btile[:, 2:3]
            h0tile = cpool.tile([U, BL_], BF16, tag="h0")
            nc.sync.dma_start(h0tile[:], h0t[:])

            for _rep in range(reps):
                xchs = {}
                ochs = {}
                pz_cur = {}
                usb_cur = {}
                pending = [None] * NS  # (ps1, ps2) for the next finalize
                half = [None] * NS
                h_prev = [h0tile[:, s * SW : (s + 1) * SW] for s in range(NS)]
                m2_prev = [None] * NS
                m1_prev = [None] * NS

                def load_chunk(k):
                    if k >= NCHUNK_ or k in xchs:
                        return
                    t0, t1x = k * TC_, (k + 1) * TC_
                    xch = xpool.tile([U, TC_, BL_], BF16, tag="xch", name=f"xch{k}")
                    nc.sync.dma_start(xch[:], xt[:, t0:t1x, :])
                    ach = apool.tile([U, TC_, BL_], BF16, tag="ach", name=f"ach{k}")
                    nc.sync.dma_start(ach[:], arep[:, t0:t1x, :])
                    xchs[k] = (xch, ach)

                def get_och(k):
                    if k not in ochs:
                        ochs[k] = opool.tile(
                            [U, TC_, BL_], BF16, tag="och", name=f"och{k}"
                        )
                    return ochs[k]

                def emit_zpair(t):
                    """Shared z gate for the step PAIR (t, t+1): one N=512
                    full-width matmul into the pair bank (even t only)."""
                    if t >= T_:
                        return
                    k, dt = divmod(t, TC_)
                    xch, _ach = xchs[k]
                    pzf = pzpool.tile([U, 2 * BL_], F32, tag="pz", name=f"pz_{t}")
                    nc.tensor.matmul(
                        pzf[:], w_z[:], xch[:, dt : dt + 2, :],
                        start=True, stop=False, skip_group_check=True,
                    )
                    pz_cur[t] = pzf
                    pz_cur[t + 1] = pzf

                def emit_xside(s, t):
                    """r/xh x-side matmuls for the step QUAD (t..t+3), emitted
                    on t%4==0: one N=512 matmul per gate per substream into a
                    [U,512] quad bank. Steps read their quarter mid-group
                    (skip_group_check; quarter-reads are chain-periods apart)."""
                    if t >= T_:
                        return
                    k, dt = divmod(t, TC_)
                    xch, _ach = xchs[k]
                    xs4 = xch[:, dt : dt + 4, s * SW : (s + 1) * SW]
                    pr = prpool.tile(
                        [U, 4 * SW], F32, tag=f"pr_{s}", name=f"pr_{s}_{t}"
                    )
                    nc.tensor.matmul(
                        pr[:], w_r[:], xs4, start=True, stop=False,
                        skip_group_check=True,
                    )
                    pxh = pxhpool.tile(
                        [U, 4 * SW], F32, tag=f"pxh_{s}", name=f"pxh_{s}_{t}"
                    )
                    nc.tensor.matmul(
                        pxh[:], w_h[:], xs4, start=True, stop=False,
                        skip_group_check=True,
                    )
                    pending[s] = (pr, pxh)

                def emit_h1(s, t):
                    """Finalize matmuls + gates + t1/uhat/m1 for step t."""
                    k, dt = divmod(t, TC_)
                    if dt == 0:
                        load_chunk(k + 1)
                        get_och(k)
                    scol = slice(s * SW, (s + 1) * SW)
                    par = t % 2
                    q = t % 4
                    prf, pxhf = pending[s]
                    pr = prf[:, q * SW : (q + 1) * SW]
                    pz = pz_cur[t][:, par * BL_ + s * SW : par * BL_ + (s + 1) * SW]
                    pzu = pz_cur[t][:, par * BL_ : (par + 1) * BL_]
                    pmm = pmmpool.tile([U, SW], F32, tag=f"pmm_{s}", name=f"pmm_{s}_{t}")
                    _xch, ach = xchs[k]

                    # h_{t-1} = m2 - m1 (both ready here): r-pair first (its
                    # bank gates the chain's sigma_r), then mm-pair (gates the
                    # off-chain copy), then z-pair.
                    hp = h_prev[s]
                    if t == 0:
                        nc.tensor.matmul(
                            pr, u_r[:], hp, start=False, stop=(q == 3),
                            skip_group_check=True,
                        )
                    else:
                        m2p, m1p = m2_prev[s], m1_prev[s]
                        nc.tensor.matmul(
                            pr, u_r[:], m2p, start=False, stop=False,
                            skip_group_check=True,
                        )
                        nc.tensor.matmul(
                            pr, un_r[:], m1p, start=False, stop=(q == 3),
                            skip_group_check=True,
                        )
                    nc.tensor.matmul(pmm[:], u_h[:], hp, start=True, stop=True)
                    nc.tensor.matmul(
                        pz, u_z[:], hp, start=False,
                        stop=(par == 1 and s == NS - 1), skip_group_check=True,
                    )

                    r_sb = wpool.tile([U, SW], BF16, tag=f"r{s}", name=f"r{s}_{t}")
                    if has_brz:
                        nc.scalar.activation(r_sb[:], pr, AF.Sigmoid, bias=b_r_ap)
                    else:
                        nc.scalar.activation(r_sb[:], pr, AF.Sigmoid)
                    if s == NS - 1:
                        # fused sigma_u over both substreams (one FD=256 op,
                        # single read of the shared z bank after its stop)
                        u_sb = wpool.tile([U, BL_], BF16, tag="usb", name=f"usb_{t}")
                        if has_brz:
                            nc.scalar.activation(
                                u_sb[:], pzu, AF.Sigmoid, bias=b_z_ap
                            )
                        else:
                            nc.scalar.activation(u_sb[:], pzu, AF.Sigmoid)
                        usb_cur[t] = u_sb

                    t1 = wpool.tile([U, SW], BF16, tag=f"t1_{s}", name=f"t1_{s}_{t}")
                    nc.vector.tensor_tensor(t1[:], pmm[:], r_sb[:], OP.mult)
                    half[s] = (t, pxhf[:, q * SW : (q + 1) * SW], t1, hp)

                def emit_h2(s):
                    """ident-MM, tanh, uhat/m1/m2, h_t, next x-side."""
                    t, pxh, t1, hp = half[s]
                    k, dt = divmod(t, TC_)
                    par = t % 2
                    scol = slice(s * SW, (s + 1) * SW)
                    och = get_och(k)
                    _xch, ach = xchs[k]

                    nc.tensor.matmul(
                        pxh, ident[:], t1[:], start=False, stop=(t % 4 == 3),
                        skip_group_check=True,
                    )

                    uhat = wpool.tile([U, SW], BF16, tag=f"uhat{s}", name=f"uhat{s}_{t}")
                    nc.vector.tensor_tensor(
                        uhat[:], usb_cur[t][:, scol], ach[:, dt, scol], OP.mult
                    )

                    htil = wpool.tile([U, SW], BF16, tag=f"htil{s}", name=f"htil{s}_{t}")
                    if has_brz:
                        nc.scalar.activation(htil[:], pxh[:], AF.Tanh, bias=b_h_ap)
                    else:
                        nc.scalar.activation(htil[:], pxh[:], AF.Tanh)

                    m2 = wpool.tile([U, SW], BF16, tag=f"m2_{s}", name=f"m2_{s}_{t}")
                    nc.vector.tensor_tensor(m2[:], uhat[:], htil[:], OP.mult)
                    m1 = wpool.tile([U, SW], BF16, tag=f"m1_{s}", name=f"m1_{s}_{t}")
                    nc.vector.scalar_tensor_tensor(
                        m1[:], uhat[:], 1.0, hp, OP.subtract, OP.mult
                    )
                    hn = och[:, dt, scol]
                    nc.vector.tensor_tensor(hn, m2[:], m1[:], OP.subtract)

                    m2_prev[s] = m2[:]
                    m1_prev[s] = m1[:]
                    h_prev[s] = hn
                    # next z-pair on odd steps; next r/xh quad on t%4==3
                    if t % 2 == 1 and s == 0:
                        emit_zpair(t + 1)
                    if t % 4 == 3:
                        emit_xside(s, t + 1)

                    if s == NS - 1 and dt == TC_ - 1:
                        nc.sync.dma_start(outt[:, k * TC_ : (k + 1) * TC_, :], och[:])
                        xchs.pop(k, None)

                load_chunk(0)
                emit_zpair(0)
                for s in range(NS):
                    emit_xside(s, 0)
                emit_h1(0, 0)
                for t in range(T_):
                    emit_h1(1, t)
                    emit_h2(0)
                    if t + 1 < T_:
                        emit_h1(0, t + 1)
                    emit_h2(1)

    nc.compile()
    return nc


def kernel(inputs, h0, W_r, U_r, b_r, W_z, U_z, b_z, W_h, U_h, b_h):
    global LAST_RESULT, LAST_IN_MAPS
    inputs = np.asarray(inputs, dtype=np.float32)
    h0 = np.asarray(h0, dtype=np.float32)
    ws = [np.asarray(w, dtype=np.float32) for w in (W_r, U_r, W_z, U_z, W_h, U_h)]
    bs = [np.asarray(b, dtype=np.float32) for b in (b_r, b_z, b_h)]

    has_brz = bool(np.any(bs[0]) or np.any(bs[1]))
    key = has_brz
    if key not in _cache:
        _cache[key] = _build(has_brz)
    nc = _cache[key]

    bf = ml_dtypes.bfloat16
    wcat = np.stack(
        [w.astype(bf) for w in ws]
        + [(-ws[1]).astype(bf), (-ws[3]).astype(bf), (-ws[5]).astype(bf)]
    )  # [9, U, U]: W_r U_r W_z U_z W_h U_h -U_r -U_z -U_h
    # reorder to W_r, U_r, W_z, U_z, W_h, U_h, -U_r, -U_z, -U_h (build order)
    ident = np.eye(U, dtype=bf)
    biases = np.stack([bs[0], bs[1], bs[2]], axis=1).astype(np.float32)  # [U, 3]

    x = inputs[:, :, :U]  # [B, T, U]
    a = inputs[:, :, U]  # [B, T]

    in_maps = []
    for c in range(NCORES):
        sl = slice(c * BL, (c + 1) * BL)
        xt_c = np.ascontiguousarray(x[sl].transpose(2, 1, 0)).astype(bf)  # [U,T,BL]
        a_tb = a[sl].T.astype(bf)  # [T, BL]
        arep_c = np.ascontiguousarray(
            np.broadcast_to(a_tb[None, :, :], (U, T, BL))
        )  # [U,T,BL]
        h0t_c = np.ascontiguousarray(h0[sl].T).astype(bf)  # [U, BL]
        in_maps.append(
            {
                "xt": xt_c,
                "arep": arep_c,
                "h0t": h0t_c,
                "wcat": wcat,
                "ident": ident,
                "biases": biases,
            }
        )

    res = run_bass_kernel_spmd(nc, in_maps, list(range(NCORES)), trace=PROFILE)
    LAST_IN_MAPS = in_maps
    LAST_RESULT = res

    out = np.empty((B, T, U), dtype=np.float32)
    for c in range(NCORES):
        sl = slice(c * BL, (c + 1) * BL)
        out[sl] = res.results[c]["outt"].astype(np.float32).transpose(2, 1, 0)
    return out

